# revision 61
# baseline (speedup 1.0000x reference)
"""GCN classifier (2x GCNConv + mean-pool + 2-layer MLP) on 8 Trainium2 cores.

Sharding strategy (graph/data parallel):
- Nodes partitioned contiguously: core c owns dst nodes [c*6250, (c+1)*6250).
- conv1: edges partitioned by dst owner, grouped into 49 windows of 128 dst
  nodes, padded to 128-edge chunks (uniform across cores -> one SPMD program).
  Host ships each core its incident edges' x rows pre-scaled by dinv[src]
  (bf16, chunk-ordered -> pure sequential DMA); scatter-add realized as
  matmuls with one-hot matrices built on-device (iota compare, one DVE pass).
- conv1 dense (W1) feature-major after PE transposes; h1 = relu(.) bf16;
  p = dinv * (h1 @ W2) node-major (carries conv2's source-side dinv).
- conv2 + mean-pool fused with NO halo exchange: conv2's output feeds only
  the per-graph mean pool, so each core pre-accumulates (on host) the tiny
  per-source pooling matrix A[s, g] = sum_{edges from s} dinv[dst] *
  onehot(batch[dst]) (+ self term). Pool partials are then just
  pg += A_w^T @ p_w over the 49 local node windows - p never leaves SBUF.
- Tail: one 64KB AllReduce of pg; mean + b2 + relu; tiny MLP replicated;
  core 0's output wins.
"""

import sys
import types

import ml_dtypes
import numpy as np

try:
    import antenv  # noqa: F401

    if "antenv.axon_hooks" not in sys.modules:
        _m = types.ModuleType("antenv.axon_hooks")
        _m._hook = None
        _m.set_axon_ntff_profile_hook = lambda h: setattr(_m, "_hook", h)
        _m.get_axon_ntff_profile_hook = lambda: _m._hook
        sys.modules["antenv.axon_hooks"] = _m
except Exception:
    pass

import concourse.bacc as bacc
import concourse.mybir as mybir
import concourse.tile as tile
from concourse import bass_utils
from concourse.masks import make_identity

F32 = mybir.dt.float32
BF16 = mybir.dt.bfloat16
F8 = mybir.dt.float8e4
AF = mybir.ActivationFunctionType
OP = mybir.AluOpType
DR = mybir.MatmulPerfMode.DoubleRow

N = 50000
E = 500000
DIN = 256
DH = 512
NG = 64
DOUT = 16

NCORES = 8
SLICE = N // NCORES  # 6250
NW = (SLICE + 127) // 128  # 49 windows
NPAD = NW * 128  # 6272
GB = 2  # windows per agg batch
NB = (NW + GB - 1) // GB  # 25
GRP = 4  # windows per dense group (512 cols)
NGRP = (NW + GRP - 1) // GRP  # 13

_COMPILED: dict = {}


def _cdiv(a, b):
    return (a + b - 1) // b


def _layout1(K1):
    """conv1 layout: per batch [w0 chunks | w1 chunks]. Returns batches, total."""
    batches = []
    gcol = 0
    for b in range(NB):
        ws = list(range(b * GB, min(NW, b * GB + GB)))
        wchunks = {w: [] for w in ws}
        rel = 0
        for w in ws:
            for _ in range(int(K1[w])):
                wchunks[w].append((gcol, rel))
                gcol += 1
                rel += 1
        batches.append((ws, wchunks, rel))
    return batches, gcol


def _preprocess(x, edge_index, batch):
    src = np.asarray(edge_index[0], dtype=np.int64)
    dst = np.asarray(edge_index[1], dtype=np.int64)
    batch = np.asarray(batch, dtype=np.int64)

    deg = np.bincount(dst, minlength=N).astype(np.float64) + 1.0
    dinv = (1.0 / np.sqrt(deg)).astype(np.float32)
    cnt = np.maximum(np.bincount(batch, minlength=NG), 1)

    # ---------- node -> (core, window, slot) assignment ----------
    # Nodes may be relabeled freely (all per-node math is slot-parallel and
    # pooling goes through A), so LPT-pack nodes into the 392 bins by edge
    # count, then group equal-load bins into the same window across cores:
    # per-window chunk counts K1 stay tight and cores stay balanced.
    import heapq

    e_n = np.bincount(dst, minlength=N).astype(np.int64)
    NHB = NW * 2  # 64-slot half-bins per core; one-hot is 64 wide
    NBINS = NCORES * NHB
    heap = [(0, b) for b in range(NBINS)]
    heapq.heapify(heap)
    slots_left = np.full(NBINS, 64, dtype=np.int64)
    fill = np.zeros(NBINS, dtype=np.int64)
    load = np.zeros(NBINS, dtype=np.int64)
    bin_of = np.empty(N, dtype=np.int64)
    slot_i = np.empty(N, dtype=np.int64)
    for n in np.argsort(-e_n, kind="stable"):
        while True:
            ld, b = heapq.heappop(heap)
            if slots_left[b] > 0:
                break
        bin_of[n] = b
        slot_i[n] = fill[b]
        fill[b] += 1
        slots_left[b] -= 1
        load[b] = ld + e_n[n]
        heapq.heappush(heap, (load[b], b))
    rank = np.empty(NBINS, dtype=np.int64)
    rank[np.argsort(-load, kind="stable")] = np.arange(NBINS)
    # group ranks (0 = heaviest) -> windows with the LIGHT groups at both
    # ends: w0 starts fast (small first DMA), w48 keeps the tail chain short
    wmap = np.empty(NW, dtype=np.int64)
    lo, hi = 0, NW - 1
    for r in range(NW - 1, -1, -1):  # lightest group first
        if (NW - 1 - r) % 2 == 0:
            wmap[r] = hi
            hi -= 1
        else:
            wmap[r] = lo
            lo += 1
    hb_rank = rank[bin_of] // NCORES  # 0..97, load-ordered half-bins
    slot_core = rank[bin_of] % NCORES
    slot_w = wmap[hb_rank // 2]
    slot_h = hb_rank % 2  # which 64-slot half of the window

    # ---------- conv1: edges grouped by (core, window, half); self-loops are
    # handled as a dense per-node term on the DVE, not in the edge stream ----
    key1 = (slot_core[dst] * NW + slot_w[dst]) * 2 + slot_h[dst]
    order1 = np.argsort(key1, kind="stable")
    ss1, ds1 = src[order1], dst[order1]
    counts1 = np.bincount(key1, minlength=NCORES * NHB).reshape(NCORES, NHB)
    starts1 = np.zeros(NCORES * NHB + 1, dtype=np.int64)
    np.cumsum(counts1.reshape(-1), out=starts1[1:])
    K1h = np.ceil(counts1.max(axis=0) / 128).astype(np.int64)  # [NHB]

    meta = (tuple(int(v) for v in K1h),)
    C1 = int(K1h.sum())
    gstart = np.zeros(NHB + 1, dtype=np.int64)
    np.cumsum(K1h, out=gstart[1:])

    # x rows pre-scaled by dinv[src] (absorbs conv1's source-side norm)
    xsc = (np.asarray(x, np.float32) * dinv[:, None]).astype(ml_dtypes.float8_e4m3)

    slot_row = slot_w * 128 + slot_h * 64 + slot_i  # node's row within core
    per_core = []
    for c in range(NCORES):
        src_cols = np.zeros((C1, 128), dtype=np.int64)
        # one-hot scatter matrices (64 wide), pre-weighted by dinv[dst]
        ohw_cols = np.zeros((C1, 128, 64), dtype=np.float32)
        cidx = np.arange(128)
        for hb in range(NHB):
            gi = c * NHB + hb
            e0, e1 = starts1[gi], starts1[gi + 1]
            n_e = int(e1 - e0)
            k = int(K1h[hb])
            sv = np.zeros(k * 128, dtype=np.int64)
            sv[:n_e] = ss1[e0:e1]
            dv = np.full(k * 128, -1, dtype=np.int64)
            dv[:n_e] = slot_i[ds1[e0:e1]]
            wv = np.zeros(k * 128, dtype=np.float32)
            wv[:n_e] = dinv[ds1[e0:e1]]
            for j in range(k):
                gcol = int(gstart[hb]) + j
                src_cols[gcol] = sv[j * 128 : (j + 1) * 128]
                dvj = dv[j * 128 : (j + 1) * 128]
                m = dvj >= 0
                ohw_cols[gcol][cidx[m], dvj[m]] = wv[j * 128 : (j + 1) * 128][m]
        x_edges = xsc[src_cols.reshape(-1)].reshape(C1, 128, DIN).transpose(1, 0, 2)
        ohw = ohw_cols.transpose(1, 0, 2).astype(ml_dtypes.float8_e4m3)
        # interleave x rows and one-hot per chunk: one DMA per window feeds
        # both matmul operands, arrivals perfectly coupled
        xoh = np.ascontiguousarray(
            np.concatenate([x_edges, ohw], axis=2).reshape(128, C1 * (DIN + 64))
        )

        mine = slot_core == c  # nodes assigned to this core
        nodes = np.nonzero(mine)[0]
        rows = slot_row[nodes]

        # own x slice (pre-scaled) at slot positions, for the self-loop term
        xs = np.zeros((NPAD, DIN), dtype=ml_dtypes.float8_e4m3)
        xs[rows] = xsc[nodes]
        xself = np.ascontiguousarray(
            xs.reshape(NW, 128, DIN).transpose(1, 0, 2).reshape(128, NW * DIN)
        )

        # pooling matrix A[slot, g] = sum_{e: src at slot} dinv[dst]*[batch[dst]=g]
        # + self: dinv[s]*[batch[s]=g]   (p rows already carry dinv[src])
        sel = mine[src]
        s_row = slot_row[src[sel]]
        d_sel = dst[sel]
        A = np.bincount(
            s_row * NG + batch[d_sel], weights=dinv[d_sel].astype(np.float64),
            minlength=NPAD * NG,
        ).reshape(NPAD, NG)
        A[rows, batch[nodes]] += dinv[nodes]
        # fold the mean-pool 1/cnt in; x256 keeps values in f8 normal range
        A *= 256.0 / np.maximum(cnt, 1)[None, :]
        A_sb = np.ascontiguousarray(
            A.reshape(NW, 128, NG).transpose(1, 0, 2).reshape(128, NW * NG)
        ).astype(ml_dtypes.float8_e4m3)

        tmp = np.ones(NPAD, dtype=np.float32)
        tmp[rows] = dinv[nodes]
        dinv_col = np.ascontiguousarray(tmp.reshape(NW, 128).T)

        per_core.append(
            dict(
                xoh=xoh,
                A=A_sb,
                dinv_col=dinv_col,
                xself=xself,
            )
        )
    return meta, per_core, cnt.astype(np.float32)


def _build_program(meta):
    (K1t,) = meta
    K1h = np.array(K1t)
    C1 = int(K1h.sum())

    nc = bacc.Bacc("TRN2", target_bir_lowering=False, debug=False, num_devices=NCORES)

    def din(name, shape, dt=F32):
        return nc.dram_tensor(name, shape, dt, kind="ExternalInput").ap()

    xoh_in = din("xoh", [128, C1 * (DIN + 64)], F8)
    xself_in = din("xself", [128, NW * DIN], F8)
    A_in = din("A", [128, NW * NG], F8)
    dinv_col_in = din("dinv_col", [128, NW])
    W1 = din("W1", [128, 2 * DH], F8)
    b1c = din("b1c", [128, DH // 128])
    W2 = din("W2", [128, 4 * (DH // 2)], F8)
    b2c = din("b2c", [128, 2])
    Wf1 = din("Wf1", [DH // 2, DH // 4])
    bf1c = din("bf1c", [128, 1])
    Wf2 = din("Wf2", [DH // 4, DOUT])
    bf2c = din("bf2c", [DOUT, 1])
    out = nc.dram_tensor("out", [NG, DOUT], F32, kind="ExternalOutput").ap()

    with tile.TileContext(nc) as tc:
        with (
            tc.tile_pool(name="const", bufs=1) as cp,
            tc.tile_pool(name="big", bufs=1) as bigp,
            tc.tile_pool(name="work", bufs=1) as wp,
            tc.tile_pool(name="psum", bufs=1, space="PSUM") as pp,
            tc.tile_pool(name="dram", bufs=1, space="DRAM") as dp,
        ):
            def load(ap_in, shape, dt=F32, pool=cp):
                t = pool.tile(shape, dt, name=ap_in.tensor.name + "_sb")
                nc.sync.dma_start(t[:], ap_in[:])
                return t

            # per-window chunk ranges: window w = half-bins 2w, 2w+1, whose
            # chunks are contiguous columns of the xoh stream
            wstart = {}
            wcount = {}
            whalf = {}
            acc_c = 0
            for w in range(NW):
                ka, kb = int(K1h[2 * w]), int(K1h[2 * w + 1])
                wstart[w] = acc_c
                wcount[w] = ka + kb
                whalf[w] = ka
                acc_c += ka + kb
            kmax = max(wcount.values())

            # -- per-window streaming: prefetch runs several windows ahead of
            # compute (and ahead of the const loads) for a smooth pipeline --
            stash = {}

            CW = DIN + 64

            def prefetch_w(w):
                if w in stash:
                    return
                c0, k = wstart[w], wcount[w]
                XO = wp.tile([128, kmax, CW], F8, tag="G1", bufs=6, name=f"xo_{w}")
                nc.sync.dma_start(
                    XO[:, :k, :].rearrange("p c d -> p (c d)"),
                    xoh_in[:, c0 * CW : (c0 + k) * CW],
                )
                stash[w] = XO

            dinv_col = load(dinv_col_in, [128, NW])

            for w in range(2):
                prefetch_w(w)

            # self-loop x rows: first windows early (needed at w0's drain)
            NWA = 6
            xselfA = cp.tile([128, NWA, DIN], F8, name="xselfA")
            nc.sync.dma_start(
                xselfA[:].rearrange("p w f -> p (w f)"), xself_in[:, : NWA * DIN]
            )

            for w in range(2, 6):
                prefetch_w(w)

            xselfB = cp.tile([128, NW - NWA, DIN], F8, name="xselfB")
            nc.sync.dma_start(
                xselfB[:].rearrange("p w f -> p (w f)"), xself_in[:, NWA * DIN :]
            )

            def xself_ap(w, h):
                p0 = 64 * h
                if w < NWA:
                    return xselfA[p0 : p0 + 64, w, :]
                return xselfB[p0 : p0 + 64, w - NWA, :]

            # consts split by first use so they don't displace the stream:
            # W1/b1/dinv at dense g0 (~w4); W2 at first p (~w5); A at pool
            W1b = cp.tile([128, 2, DH], F8, name="W1b")
            nc.sync.dma_start(W1b[:].rearrange("p k f -> p (k f)"), W1[:])
            b1_sb = load(b1c, [128, DH // 128])
            idbf = cp.tile([128, 128], BF16)
            make_identity(nc, idbf[:])
            W2b = cp.tile([128, 4, DH // 2], F8, name="W2b")
            nc.sync.dma_start(W2b[:].rearrange("p k f -> p (k f)"), W2[:])
            A_sb = cp.tile([128, NW, NG], F8, name="A_sb")
            nc.sync.dma_start(A_sb[:].rearrange("p w g -> p (w g)"), A_in[:])
            b2_sb = load(b2c, [128, 2])

            for w in range(6, 9):
                prefetch_w(w)

            # tail-only consts
            bf1_sb = load(bf1c, [128, 1])
            bf2_sb = load(bf2c, [DOUT, 1])
            Wf1_sb = [cp.tile([128, DH // 4], F32, name=f"wf1_{k}") for k in range(2)]
            for k in range(2):
                nc.sync.dma_start(Wf1_sb[k][:], Wf1[k * 128 : (k + 1) * 128, :])
            Wf2_sb = cp.tile([128, DOUT], F32)
            nc.sync.dma_start(Wf2_sb[:], Wf2[:])
            idf32 = cp.tile([128, 128], F32)
            make_identity(nc, idf32[:])

            # h1 feature-major, k-tile pairs interleaved for DoubleRow
            h1p = [bigp.tile([128, 2, NPAD], F8, name=f"h1p_{t}") for t in range(2)]
            sfm_groups: dict = {}

            def sfm_of(g):
                if g not in sfm_groups:
                    sfm_groups[g] = wp.tile(
                        [128, 2, GRP * 128], F8, tag="sfm", bufs=2, name=f"sfm_{g}"
                    )
                return sfm_groups[g]

            g_local = dp.tile([NG, DH // 2], F32)
            g_red = dp.tile([NG, DH // 2], F32, addr_space="Shared")

            pg = pp.tile([NG, DH // 2], F32, tag="pool", bufs=1, name="pg")

            # ---- conv1 aggregation, one dst window at a time ----
            # ohw carries dinv[dst]; x rows carry dinv[src]; chunk pairs run
            # as 256-deep fp8 DoubleRow matmuls. Stored agg is scaled x8 to
            # sit in f8's normal range.
            def emit_agg_window(w):
                if w not in stash:
                    prefetch_w(w)
                XO = stash.pop(w)
                snm = wp.tile([128, DIN], BF16, tag="snm", bufs=2, name=f"snm_{w}")
                for h in range(2):
                    o0, k = (0, whalf[w]) if h == 0 else (whalf[w], wcount[w] - whalf[w])
                    acc = pp.tile(
                        [64, DIN], F32, tag="agg64", bufs=2, name=f"acc1_{w}_{h}"
                    )
                    npair = k // 2
                    nmm = npair + (k & 1)
                    for j in range(npair):
                        c = o0 + 2 * j
                        nc.tensor.matmul(
                            out=acc[:],
                            lhsT=XO[:, c : c + 2, DIN:],
                            rhs=XO[:, c : c + 2, :DIN],
                            start=(j == 0),
                            stop=(j == nmm - 1),
                            perf_mode=DR,
                        )
                    if k & 1:
                        nc.tensor.matmul(
                            out=acc[:],
                            lhsT=XO[:, o0 + k - 1, DIN:],
                            rhs=XO[:, o0 + k - 1, :DIN],
                            start=(npair == 0),
                            stop=True,
                        )
                    # self-loop term folded into the drain: acc + dinv*xself
                    nc.vector.scalar_tensor_tensor(
                        out=snm[64 * h : 64 * h + 64, :],
                        in0=xself_ap(w, h),
                        scalar=dinv_col[64 * h : 64 * h + 64, w : w + 1],
                        in1=acc[:],
                        op0=OP.mult,
                        op1=OP.add,
                    )
                sf = sfm_of(w // GRP)
                wc = (w % GRP) * 128
                for k2 in range(2):
                    pt = pp.tile([128, 128], BF16, tag="t", bufs=1, name=f"pt_{w}_{k2}")
                    nc.tensor.transpose(pt[:], snm[:, k2 * 128 : (k2 + 1) * 128], idbf[:])
                    nc.vector.tensor_scalar(
                        out=sf[:, k2, wc : wc + 128], in0=pt[:],
                        scalar1=8.0, scalar2=None, op0=OP.mult,
                    )
                return snm

            # ---- conv1 dense (h1 = relu(agg @ W1 + b1), feature-major) ----
            # sf = 8*agg (f8), W1 = 64*W1 (f8) -> ph = 512*(agg@W1); stored
            # h1p = 8*h1 via drain scale 1/64 (b1c is host-scaled x8).
            def emit_dense_group(g):
                c0 = g * GRP * 128
                cw = min(GRP * 128, NPAD - c0)
                sf = sfm_of(g)
                for m in range(4):
                    ph = pp.tile([128, GRP * 128], F32, tag="h1", bufs=2, name=f"ph1_{g}_{m}")
                    nc.tensor.matmul(
                        out=ph[:, :cw],
                        lhsT=W1b[:, :, m * 128 : (m + 1) * 128],
                        rhs=sf[:, :, :cw],
                        start=True,
                        stop=True,
                        perf_mode=DR,
                    )
                    nc.scalar.activation(
                        h1p[m // 2][:, m % 2, c0 : c0 + cw],
                        ph[:, :cw],
                        AF.Relu,
                        bias=b1_sb[:, m : m + 1],
                        scale=1.0 / 64,
                    )

            # ---- p = dinv * (h1 @ W2); pool pairs pg += A_w^T @ p_w (DR) ----
            # ppm = 512*(h1@W2); stored pb = 16*p; A = 256*A/cnt; the
            # accumulated pg = 4096*mean is descaled in the tail drain.
            pb_pair = {}

            def emit_p_pool(w):
                c0 = w * 128
                ppm = pp.tile([128, DH // 2], F32, tag="agg", bufs=2, name=f"pp_{w}")
                for t in range(2):
                    nc.tensor.matmul(
                        out=ppm[:],
                        lhsT=h1p[t][:, :, c0 : c0 + 128],
                        rhs=W2b[:, 2 * t : 2 * t + 2, :],
                        start=(t == 0),
                        stop=(t == 1),
                        perf_mode=DR,
                    )
                if w == NW - 1:
                    pbl = wp.tile([128, DH // 2], F8, tag="pbl", bufs=1, name="pb_last")
                    nc.vector.tensor_scalar(
                        out=pbl[:], in0=ppm[:], scalar1=dinv_col[:, w : w + 1],
                        scalar2=1.0 / 32, op0=OP.mult, op1=OP.mult,
                    )
                    nc.tensor.matmul(
                        out=pg[:], lhsT=A_sb[:, w, :], rhs=pbl[:],
                        start=False, stop=True,
                    )
                    return
                if w % 2 == 0:
                    pb_pair[w // 2] = wp.tile(
                        [128, 2, DH // 2], F8, tag="pb", bufs=2, name=f"pb_{w}"
                    )
                pb = pb_pair[w // 2]
                nc.vector.tensor_scalar(
                    out=pb[:, w % 2, :], in0=ppm[:], scalar1=dinv_col[:, w : w + 1],
                    scalar2=1.0 / 32, op0=OP.mult, op1=OP.mult,
                )
                if w % 2 == 1:
                    nc.tensor.matmul(
                        out=pg[:],
                        lhsT=A_sb[:, w - 1 : w + 1, :],
                        rhs=pb[:],
                        start=(w == 1),
                        stop=False,
                        perf_mode=DR,
                    )

            # spread dense/p/pool between agg windows so the PE queue never
            # bursts a dependent chain at group boundaries
            after = {w: [] for w in range(NW)}
            for g in range(NGRP):
                after[min(4 * g + GRP, NW - 1)].append(("d", g))
                for i in range(GRP):
                    w2 = 4 * g + i
                    if w2 >= NW:
                        break
                    after[min(4 * g + GRP + 1 + i, NW - 1)].append(("p", w2))

            bar_in = dp.tile([1, 1], BF16)
            bar_out = dp.tile([NCORES, 1], BF16, addr_space="Shared")
            for w in range(NW):
                if w + 5 < NW:
                    prefetch_w(w + 5)
                snm_w = emit_agg_window(w)
                if w == 34:
                    # pre-sync: the dummy-write gives the barrier a data dep
                    # on window 34's compute, so it triggers when each core is
                    # ~70% done - absorbing inter-core skew on the idle CC
                    # path - and still clears the CC stream well before the
                    # final AllReduce
                    nc.gpsimd.dma_start(bar_in[:], snm_w[0:1, 0:1])
                    nc.gpsimd.collective_compute(
                        "AllGather",
                        OP.bypass,
                        replica_groups=[list(range(NCORES))],
                        ins=[bar_in.opt()],
                        outs=[bar_out.opt()],
                    )
                if w < NW - 1:
                    for kind, v in after[w]:
                        if kind == "d":
                            emit_dense_group(v)
                        else:
                            emit_p_pool(v)
            for kind, v in after[NW - 1]:
                if kind == "d":
                    emit_dense_group(v)
                else:
                    emit_p_pool(v)

            # ---------------- tail: AllReduce + mean + relu + MLP ----------------
            gsb = wp.tile([NG, DH // 2], F32)
            nc.vector.tensor_copy(gsb[:], pg[:])
            nc.sync.dma_start(g_local[:], gsb[:])
            nc.gpsimd.collective_compute(
                "AllReduce",
                OP.add,
                replica_groups=[list(range(NCORES))],
                ins=[g_local.opt()],
                outs=[g_red.opt()],
            )
            gsum = wp.tile([NG, DH // 2], F32)
            nc.sync.dma_start(gsum[:], g_red[:])

            # transpose to feature-major, then relu(gsum/4096 + b2) on the
            # drain (b2 is per-feature = per-partition after the transpose)
            g_fm = [wp.tile([128, NG], F32, name=f"gfm_{k}") for k in range(2)]
            for k in range(2):
                pt = pp.tile([128, NG], F32, tag="t", bufs=1, name=f"gt_{k}")
                nc.tensor.transpose(pt[:], gsum[:, k * 128 : (k + 1) * 128], idf32[:NG, :NG])
                nc.scalar.activation(
                    g_fm[k][:], pt[:], AF.Relu, bias=b2_sb[:, k : k + 1], scale=1.0 / 4096
                )
            pz = pp.tile([128, NG], F32, tag="h1", bufs=2, name="pz")
            for k in range(2):
                nc.tensor.matmul(
                    out=pz[:], lhsT=Wf1_sb[k][:], rhs=g_fm[k][:], start=(k == 0), stop=(k == 1)
                )
            zsb = wp.tile([128, NG], F32)
            nc.scalar.activation(zsb[:], pz[:], AF.Relu, bias=bf1_sb[:, 0:1])
            po = pp.tile([DOUT, NG], F32, tag="t", bufs=1, name="po")
            nc.tensor.matmul(out=po[:], lhsT=Wf2_sb[:], rhs=zsb[:], start=True, stop=True)
            osb = wp.tile([DOUT, NG], F32)
            nc.scalar.activation(osb[:], po[:], AF.Relu, bias=bf2_sb[:, 0:1])
            pout = pp.tile([NG, DOUT], F32, tag="t", bufs=1, name="pout")
            nc.tensor.transpose(pout[:], osb[:], idf32[:DOUT, :DOUT])
            out_sb = wp.tile([NG, DOUT], F32)
            nc.vector.tensor_copy(out_sb[:], pout[:])
            nc.sync.dma_start(out[:], out_sb[:])

    nc.compile()
    return nc


def _get_program(meta):
    if meta not in _COMPILED:
        _COMPILED[meta] = _build_program(meta)
    return _COMPILED[meta]


def _make_in_maps(W1, b1, W2, b2, Wf1, bf1, Wf2, bf2, per_core, cnt):
    W1a = np.asarray(W1, np.float32).reshape(2, 128, DH).transpose(1, 0, 2)
    W2a = np.asarray(W2, np.float32).reshape(4, 128, DH // 2).transpose(1, 0, 2)
    shared = dict(
        W1=np.ascontiguousarray(W1a.reshape(128, 2 * DH) * 64).astype(
            ml_dtypes.float8_e4m3
        ),
        b1c=np.ascontiguousarray(
            np.asarray(b1, np.float32).reshape(DH // 128, 128).T * 8
        ),
        W2=np.ascontiguousarray(W2a.reshape(128, 4 * (DH // 2)) * 64).astype(
            ml_dtypes.float8_e4m3
        ),
        b2c=np.ascontiguousarray(np.asarray(b2, np.float32).reshape(2, 128).T),
        Wf1=np.asarray(Wf1, np.float32),
        bf1c=np.asarray(bf1, np.float32).reshape(DH // 4, 1),
        Wf2=np.asarray(Wf2, np.float32),
        bf2c=np.asarray(bf2, np.float32).reshape(DOUT, 1),
    )
    return [dict(shared, **per_core[c]) for c in range(NCORES)]


def kernel(
    x, W1, b1, W2, b2, Wf1, bf1, Wf2, bf2, edge_index, batch, num_graphs, _trace=False
):
    assert int(num_graphs) == NG
    meta, per_core, cnt = _preprocess(
        np.asarray(x), np.asarray(edge_index), np.asarray(batch)
    )
    nc = _get_program(meta)
    in_maps = _make_in_maps(W1, b1, W2, b2, Wf1, bf1, Wf2, bf2, per_core, cnt)
    res = bass_utils.run_bass_kernel_spmd(
        nc, in_maps, core_ids=list(range(NCORES)), trace=_trace
    )
    out = np.asarray(res.results[0]["out"], np.float32)
    if _trace:
        kernel._last_results = res
    return out
